# revision 53
# baseline (speedup 1.0000x reference)
"""Joint-entropy (KDE logsumexp over 3x3 windows) Trainium2 kernel.

Math: for each 3x3 window of pixel vectors v_n (C=3 channels),
  out[i,j] = log_norm - (1/9) * sum_n log(S_n),  S_n = sum_m exp(-2*||v_n-v_m||^2)

Per-pair Gaussians via Act's Derivative_Erf: derf(sqrt(2)*d) =
(2/sqrt(pi))*exp(-2 d^2), so prod_c derf = k*exp(-2||d||^2), k=(2/sqrt(pi))^3.
Every E value carries k; the self term "+1" becomes "+k" (folded into the
Alo combos and the PSUM-copy biases) and the final affine adds ln(k):
  out = (log_norm + ln k) - (1/9) * ln prod_n (k + sum_{m!=n} kE_nm).

Sharding: 8 cores = 4 batches x 2 row-halves; host-prepped bf16 slab
[130, 3, 264] per core (129 rows + 1 pad row, width padded 4 each side);
output [127, 254] f32 per core, no collectives.  On chip, partitions =
image rows; three row-shifted views Xs[p] = row p+s (separate DMAs of one
DRAM slab, so compute starts when its views land) make every row gap
reachable with partition-0-aligned operands (a hard ISA requirement for
compute engines).  All DMAs that matter ride queues/directions measured
to fan out across the 16 DMA engines.

E classes (one bf16 DVE sub -> one Act Derivative_Erf -> 2 DVE channel
muls, wide over the b-planes via stride tricks), anchor cols -2..257:
  M1 [128,5,260] a=1 (X0 vs X1), M2 [128,5,260] a=2 (X0 vs X2; row 127 is
  junk from the pad row, never consumed), ME0/ME1 [128,2,260] a=0 at row
  alignments 0..127 / 1..128.

Stage C: sliding 3-sums over b (D = down-pair sums at col +0, U = up-pair
sums with per-plane column shifts encoded in the plane stride), same-row
A combos.  Y = Ahi + U1 lets the role sums be S0 = Alo+D1+D2,
S1 = Y + D1@+1, S2 = Y@+1 + U2, so only two partition shifts remain;
they run on the otherwise-idle PE as matmuls with an off-diagonal
identity (out[m] = in[m+1]), and the PSUM->SBUF casts ride the Act
engine with bias=+k.  Role sums use diagonal plane-stride reads (plane
nc at col +nc) so each is a single wide 2x bf16 add.

Stage D: 4 muls (tree product of the 9 S maps), Ln on Act, final affine
on DVE; output split across both HWDGE queues (SP/Act), which keeps the
slow SWDGE drain off the exit path.
"""

import dataclasses

import numpy as np
import ml_dtypes

import concourse.bacc as bacc
import concourse.tile as tile
from concourse import mybir
from concourse.bass_utils import run_bass_kernel_spmd

F32 = mybir.dt.float32
BF16 = mybir.dt.bfloat16
AOP = mybir.AluOpType
AF = mybir.ActivationFunctionType

B = 4
C = 3
W = 256
PAD = 4           # host zero-pad each side
WT = W + 2 * PAD  # 264
WA = W + 4        # 260: anchor cols -2..257
ROWS_IN = 129
P = 128
POUT = 127
WOUT = 254
SROW = C * WT     # one input row in elements (792)
SQRT2 = float(np.sqrt(2.0))
CDERF = float(2.0 / np.sqrt(np.pi))
K = CDERF**3
LOG_NORM = float(np.log(9.0) + 3.0 * np.log(np.sqrt(2.0 * np.pi) * 0.5))
AFFINE_C = LOG_NORM + 3.0 * float(np.log(CDERF))


def _with_dims(base_ap, dims):
    """Replace free dims of `base_ap` (partition dim kept) with the given
    [stride, count] pairs (strides in elements)."""
    ap = [list(base_ap.ap[0])] + [list(d) for d in dims]
    return dataclasses.replace(base_ap, ap=ap)


def _build_program():
    nc = bacc.Bacc("TRN2")
    xin = nc.dram_tensor("xin", (ROWS_IN + 1, C, WT), BF16, kind="ExternalInput")
    wsh = nc.dram_tensor("wshift", (P, P), BF16, kind="ExternalInput")
    yout = nc.dram_tensor("yout", (POUT, WOUT), F32, kind="ExternalOutput")

    with tile.TileContext(nc) as tc:
        with (
            tc.tile_pool(name="p", bufs=1) as tp,
            tc.tile_pool(name="pp", bufs=1, space="PSUM") as pp,
        ):
            def ap_of(base, elem_off, dims):
                return dataclasses.replace(
                    _with_dims(base, dims), offset=base.offset + elem_off
                )

            # ---- load Xs[p] = input row p+s, one DMA per shift ----------
            XS = []
            for s in range(3):
                xt = tp.tile([P, C, WT], BF16, tag=f"x{s}")
                nc.gpsimd.dma_start(
                    out=xt, in_=ap_of(xin[0:P, :, :], s * SROW, [[WT, C], [1, WT]])
                )
                XS.append(xt)
            X0, X1, X2 = XS
            WS = tp.tile([P, P], BF16, tag="ws")
            nc.sync.dma_start(out=WS, in_=wsh[:, :])

            ME0 = tp.tile([P, 2, WA], BF16, tag="me0")  # a=0 rows 0..127
            ME1 = tp.tile([P, 2, WA], BF16, tag="me1")  # a=0 rows 1..128

            def cls(tag, xa, xb, nb, out_ap):
                """E class: anchor `xa` bcast over b; other `xb` at col +b;
                both [P, C, *]-shaped slice APs. Writes k*exp(-2 d2) planes."""
                anchor = _with_dims(xa, [[0, nb], [WT, C], [1, WA]])
                other = _with_dims(xb, [[1, nb], [WT, C], [1, WA]])
                d = tp.tile([P, nb, C, WA], BF16, tag=f"d_{tag}")
                nc.vector.tensor_sub(d, anchor, other)
                g = tp.tile([P, nb, C, WA], BF16, tag=f"g_{tag}")
                nc.scalar.activation(g, d, AF.Derivative_Erf, scale=SQRT2)
                g01 = tp.tile([P, nb, WA], BF16, tag=f"g01_{tag}")
                nc.vector.tensor_mul(g01, g[:, :, 0, :], g[:, :, 1, :])
                nc.vector.tensor_mul(out_ap, g01, g[:, :, 2, :])

            def d_combo(mt, tag):
                t4 = tp.tile([P, 4, W], BF16, tag=f"t4{tag}")
                nc.vector.tensor_add(t4, mt[:, 0:4, 2 : 2 + W], mt[:, 1:5, 2 : 2 + W])
                out = tp.tile([P, 3, W], BF16, tag=f"dc{tag}")
                nc.vector.tensor_add(out, t4[:, 0:3, :], mt[:, 2:5, 2 : 2 + W])
                return out

            def u_combo(mt, tag):
                # plane t = sum_{j=t..t+2} mt[:, j, col + 4 - j]
                t4 = tp.tile([P, 4, W], BF16, tag=f"u4{tag}")
                in0 = ap_of(mt[0:P, 0, 0:W], 4, [[WA - 1, 4], [1, W]])
                in1 = ap_of(mt[0:P, 0, 0:W], WA + 3, [[WA - 1, 4], [1, W]])
                nc.vector.tensor_add(t4, in0, in1)
                out = tp.tile([P, 3, W], BF16, tag=f"uc{tag}")
                in2 = ap_of(mt[0:P, 0, 0:W], 2 * WA + 2, [[WA - 1, 3], [1, W]])
                nc.vector.tensor_add(out, t4[:, 0:3, :], in2)
                return out

            # ---- e0lo first (only needs X0), then the a=1 class ---------
            cls("e0lo", X0[0:P, :, 2 : 2 + WA], X0[0:P, :, 3 : 3 + WA], 2, ME0)
            M1 = tp.tile([P, 5, WA], BF16, tag="m1")
            cls("m1", X0[0:P, :, 2 : 2 + WA], X1[0:P, :, 0:WA], 5, M1)
            D1 = d_combo(M1, "d1")
            U1 = u_combo(M1, "u1")

            # Partition shift by 1 on the (otherwise idle) PE:
            # out[m, :] = sum_k WS[k, m] * in[k, :] with WS = eye(k=-1),
            # i.e. out[m] = in[m+1].  Output lands in PSUM as f32.
            def pshift(t, tag):
                o = pp.tile([P, C * W], F32, tag=f"sh{tag}")
                rhs0 = ap_of(t[0:P, 0, 0:W], 0, [[1, 512]])
                nc.tensor.matmul(o[:, 0:512], WS, rhs0, start=True, stop=True)
                rhs1 = ap_of(t[0:P, 0, 0:W], 512, [[1, 256]])
                nc.tensor.matmul(o[:, 512:768], WS, rhs1, start=True, stop=True)
                return o

            D1h = pshift(D1, "d1")

            # ---- remaining a=0 class, A combos, Y -----------------------
            cls("e0hi", X1[0:P, :, 2 : 2 + WA], X1[0:P, :, 3 : 3 + WA], 2, ME1)

            # A combos per row alignment (pair sums of a=0 E maps):
            #  nc=0: E01(j)+E02(j); nc=1: E01(j-1)+E01(j); nc=2: E02(j-2)+E01(j-1)
            # The self term +k rides Alo only (S1/S2 get it via the PSUM
            # copy bias); Ahi stays on the fast tensor_add path.
            pairs = [((0, 2), (1, 2)), ((0, 1), (0, 2)), ((1, 0), (0, 1))]

            Ahi = tp.tile([P, 3, W], BF16, tag="ahi")
            for ncol, ((k0, o0), (k1, o1)) in enumerate(pairs):
                nc.vector.tensor_add(
                    Ahi[:, ncol, :],
                    ME1[0:P, k0, o0 : o0 + W],
                    ME1[0:P, k1, o1 : o1 + W],
                )

            Y = tp.tile([P, 3, W], BF16, tag="y")
            nc.vector.tensor_add(Y, Ahi, U1)

            # ---- a=2 class + combos; Y shift on PE ----------------------
            M2 = tp.tile([P, 5, WA], BF16, tag="m2")
            cls("m2", X0[0:P, :, 2 : 2 + WA], X2[0:P, :, 0:WA], 5, M2)

            Yh = pshift(Y, "y")

            Alo = tp.tile([P, 3, W], BF16, tag="alo")
            for ncol, ((k0, o0), (k1, o1)) in enumerate(pairs):
                nc.vector.scalar_tensor_tensor(
                    out=Alo[:, ncol, :],
                    in0=ME0[0:P, k0, o0 : o0 + W],
                    scalar=K,
                    in1=ME0[0:P, k1, o1 : o1 + W],
                    op0=AOP.add,
                    op1=AOP.add,
                )

            D2 = d_combo(M2, "d2")
            U2 = u_combo(M2, "u2")

            # PSUM -> SBUF copies on the idle Act engine; bias adds the +k
            # self term for S1/S2 and the cast restores 2x DVE reads.
            KB = tp.tile([P, 1], F32, tag="kb")
            nc.gpsimd.memset(KB, K)
            D1c = tp.tile([P, C * W], BF16, tag="d1c")
            nc.scalar.activation(D1c, D1h, AF.Identity, bias=KB[:, :])
            Yc = tp.tile([P, C * W], BF16, tag="yc")
            nc.scalar.activation(Yc, Yh, AF.Identity, bias=KB[:, :])

            # ---- role sums [127, 3(nc), 254] ----------------------------
            def diag(base, plane0, dplane):
                """[POUT, 3, WOUT] view: plane i at (plane0 + i*dplane, col+i);
                base = [POUT, WOUT] slice of a W-plane-stride tile."""
                return ap_of(base, plane0 * W, [[dplane * W + 1, 3], [1, WOUT]])

            S0 = tp.tile([POUT, 3, WOUT], BF16, tag="s0")
            nc.vector.tensor_add(
                S0, diag(Alo[0:POUT, 0, 0:WOUT], 0, 1),
                diag(D1[0:POUT, 0, 0:WOUT], 2, -1),
            )
            nc.vector.tensor_add(S0, S0, diag(D2[0:POUT, 0, 0:WOUT], 2, -1))
            S1 = tp.tile([POUT, 3, WOUT], BF16, tag="s1")
            nc.vector.tensor_add(
                S1, diag(Y[0:POUT, 0, 0:WOUT], 0, 1),
                diag(D1c[0:POUT, 0:WOUT], 2, -1),
            )
            S2 = tp.tile([POUT, 3, WOUT], BF16, tag="s2")
            nc.vector.tensor_add(
                S2, diag(Yc[0:POUT, 0:WOUT], 0, 1),
                diag(U2[0:POUT, 0, 0:WOUT], 0, 1),
            )

            # ---- stage D: product, log, affine --------------------------
            T0 = tp.tile([POUT, 3, WOUT], BF16, tag="t0")
            nc.vector.tensor_mul(T0, S0, S1)
            T1 = tp.tile([POUT, 3, WOUT], BF16, tag="t1")
            nc.vector.tensor_mul(T1, T0, S2)
            R = tp.tile([POUT, WOUT], BF16, tag="r")
            nc.vector.tensor_mul(R, T1[:, 0, :], T1[:, 1, :])
            PP = tp.tile([POUT, WOUT], BF16, tag="pp")
            nc.vector.tensor_mul(PP, R, T1[:, 2, :])
            # Fold the affine constant into the Ln input scale:
            #   -(1/9)*ln(PP * exp(-9*AFFINE_C)) = AFFINE_C - (1/9)*ln(PP)
            # so the tail is Ln -> scale-only Identity, both on Act with the
            # out1 DMA gen following on the same queue (no cross-engine hops).
            L = tp.tile([POUT, WOUT], F32, tag="lnp")
            nc.scalar.activation(
                L, PP, AF.Ln, scale=float(np.exp(-9.0 * AFFINE_C))
            )
            OUT = tp.tile([POUT, WOUT], F32, tag="out")
            nc.scalar.activation(OUT, L, AF.Copy, scale=-1.0 / 9.0)
            nc.sync.dma_start(out=yout[0:64, :], in_=OUT[0:64, :])
            nc.scalar.dma_start(out=yout[64:POUT, :], in_=OUT[64:POUT, :])
    if not nc.is_finalized():
        nc.finalize()
    return nc


_PROGRAM = None


def _get_program():
    global _PROGRAM
    if _PROGRAM is None:
        _PROGRAM = _build_program()
    return _PROGRAM


def _shard_inputs(x):
    x = np.asarray(x, dtype=np.float32)
    xp = np.zeros((B, 257, C, WT), dtype=ml_dtypes.bfloat16)
    xp[:, :256, :, PAD : PAD + W] = np.transpose(x, (0, 2, 1, 3))
    ws = np.eye(P, k=-1, dtype=ml_dtypes.bfloat16)
    in_maps = []
    for core in range(8):
        b, half = divmod(core, 2)
        r0 = half * POUT
        in_maps.append(
            {"xin": np.ascontiguousarray(xp[b, r0 : r0 + ROWS_IN + 1]), "wshift": ws}
        )
    return in_maps


def _gather(results):
    out = np.empty((B, 254, 254), dtype=np.float32)
    for core in range(8):
        b, half = divmod(core, 2)
        out[b, half * POUT : half * POUT + POUT, :] = results[core]["yout"]
    return out


def kernel(x, **_unused):
    nc = _get_program()
    res = run_bass_kernel_spmd(nc, _shard_inputs(x), core_ids=list(range(8)))
    return _gather(res.results)


def kernel_traced(x):
    """Same as kernel() but returns (output, BassKernelResults) with trace."""
    nc = _get_program()
    res = run_bass_kernel_spmd(
        nc, _shard_inputs(x), core_ids=list(range(8)), trace=True
    )
    return _gather(res.results), res
